# revision 22
# baseline (speedup 1.0000x reference)
"""ColumnParallelLinear kernel for Trainium2 (8 NeuronCores).

Computes Y[s,b,o] = sum_h X[s,b,h] * W[o,h]  (F.linear / einsum 'sbh,oh->sbo')
with S,B,H,OUT = 2048,4,1024,4096, fp32.

Strategy:
  - Flatten tokens: M = S*B = 8192 rows.  GEMM: [M,H] @ [H,OUT].
  - 2D shard over 8 cores: 4 token groups (2048 rows) x 2 out-column
    groups (2048 cols).  This minimizes per-core HBM traffic
    (x 8.4MB + w 8.4MB + y 16.8MB = 33.6MB/core) vs pure column- or
    row-parallel, keeping the kernel compute-bound.
  - Host packs X and W into [chunk][partition][k][free] layout so every
    DMA descriptor moves a 16KB contiguous run (DMA is descriptor-rate
    limited at small runs); y is written as full 8KB rows.
  - Matmuls run as float32r (fp32 bits, full-rate PE path; moving dim
    512), accumulating fp32 in PSUM.
"""

import numpy as np

import concourse.bass as bass
from concourse import bacc
import concourse.mybir as mybir
import concourse.tile as tile
from concourse.bass_utils import run_bass_kernel_spmd

S, B, H, OUT = 2048, 4, 1024, 4096
M = S * B

N_CORES = 8
G_ROW, G_COL = 4, 2          # token groups x out-feature groups
M_LOC = M // G_ROW           # 2048 rows per core
N_LOC = OUT // G_COL         # 2048 out features per core

P = 128
KO = H // P                  # 8 contraction subtiles
NT = 512                     # psum free dim (one fp32 bank)
NO = N_LOC // NT             # 4 col tiles
XG = 512                     # x chunk width (4 row tiles)
NXG = M_LOC // XG            # 4 chunks
MO = M_LOC // P              # 16 row tiles

MM_DT = mybir.dt.float32r    # full-rate fp32 matmul path


def build_nc(mm_dt=MM_DT):
    nc = bacc.Bacc(None, target_bir_lowering=False, enable_partition_id=False)
    # packed inputs: [chunk][partition p][k][free] so each partition's slice
    # of one chunk is 16KB contiguous in DRAM (one descriptor per partition)
    xH = nc.declare_dram_parameter("xH", [NXG, P, KO, XG], mybir.dt.float32,
                                   isOutput=False)
    wH = nc.declare_dram_parameter("wH", [NO, P, KO, NT], mybir.dt.float32,
                                   isOutput=False)
    y = nc.declare_dram_parameter("y", [M_LOC, N_LOC], mybir.dt.float32,
                                  isOutput=True)
    y_r = y[:, :].rearrange("(mo p) n -> p mo n", p=P)

    with tile.TileContext(nc) as tc:
        with (
            tc.tile_pool(name="xp", bufs=1) as xp,
            tc.tile_pool(name="wp", bufs=1) as wp,
            tc.tile_pool(name="op", bufs=2) as op,
            tc.tile_pool(name="psp", bufs=8, space="PSUM") as psp,
        ):
            x_sb = [None] * NXG
            w_sb = [None] * NO
            KQ = 2  # k-pair granularity for the startup-critical chunks

            def load_x(g, split=False):
                if split:
                    x_sb[g] = ("split", [None] * (KO // KQ))
                else:
                    t = xp.tile([P, KO, XG], mm_dt, tag=f"x{g}", name=f"x{g}")
                    nc.sync.dma_start(t[:], xH[g, :, :, :].bitcast(mm_dt))
                    x_sb[g] = ("whole", t)

            def load_w(n, split=False):
                if split:
                    w_sb[n] = ("split", [None] * (KO // KQ))
                else:
                    t = wp.tile([P, KO, NT], mm_dt, tag=f"w{n}", name=f"w{n}")
                    nc.sync.dma_start(t[:], wH[n, :, :, :].bitcast(mm_dt))
                    w_sb[n] = ("whole", t)

            def load_w_quarters(n, qs=None):
                for q in (qs if qs is not None else range(KO // KQ)):
                    t = wp.tile([P, KQ, NT], mm_dt, tag=f"w{n}q{q}",
                                name=f"w{n}q{q}")
                    nc.sync.dma_start(
                        t[:], wH[n, :, q * KQ:(q + 1) * KQ, :].bitcast(mm_dt)
                    )
                    w_sb[n][1][q] = t

            def load_x_quarters(g, qs=None):
                for q in (qs if qs is not None else range(KO // KQ)):
                    t = xp.tile([P, KQ, XG], mm_dt, tag=f"x{g}q{q}",
                                name=f"x{g}q{q}")
                    nc.sync.dma_start(
                        t[:], xH[g, :, q * KQ:(q + 1) * KQ, :].bitcast(mm_dt)
                    )
                    x_sb[g][1][q] = t

            def slice_k(entry, k, lo, hi):
                kind, t = entry
                if kind == "whole":
                    return t[:, k, lo:hi]
                return t[k // KQ][:, k % KQ, lo:hi]

            # arrival order matches consumption: g0 runs n-outer so it only
            # needs w_n just-in-time while the rest streams in; the first
            # chunk pair is k-split so the PE ramps during arrival
            load_w(0, split=True)
            load_x(0, split=True)
            load_w_quarters(0)
            load_x_quarters(0)
            load_w(1)
            load_w(2)
            load_w(3)
            load_x(1)
            load_x(2)
            load_x(3)

            def do_group(g, n_outer, tail=False):
                stages = [op.tile([P, N_LOC], mybir.dt.float32, tag=f"st{mi}",
                                  name=f"st{g}_{mi}")
                          for mi in range(XG // P)]
                outer = range(NO) if n_outer else range(XG // P)
                inner = range(XG // P) if n_outer else range(NO)
                for a in outer:
                    for b in inner:
                        n, mi = (a, b) if n_outer else (b, a)
                        ps = psp.tile([P, NT], mybir.dt.float32)
                        for k in range(KO):
                            nc.tensor.matmul(
                                ps[:],
                                lhsT=slice_k(x_sb[g], k, mi * P, (mi + 1) * P),
                                rhs=slice_k(w_sb[n], k, 0, NT),
                                start=(k == 0),
                                stop=(k == KO - 1),
                            )
                        nc.vector.tensor_copy(
                            stages[mi][:, n * NT:(n + 1) * NT], ps[:]
                        )
                        if tail and mi == XG // P - 1:
                            # final stage: per-n writes right after each copy
                            # so the last write trails the last matmul by as
                            # little as possible
                            mo = g * (XG // P) + mi
                            nc.scalar.dma_start(
                                y_r[:, mo, n * NT:(n + 1) * NT],
                                stages[mi][:, n * NT:(n + 1) * NT],
                            )
                # full 8KB-run row writes on the ACT HWDGE ring
                last = XG // P - (1 if tail else 0)
                for mi in range(last):
                    mo = g * (XG // P) + mi
                    nc.scalar.dma_start(y_r[:, mo, :], stages[mi][:])

            do_group(0, n_outer=True)     # w arrives n-by-n
            for g in range(1, NXG):
                # mi-outer spreads the writes
                do_group(g, n_outer=False, tail=(g == NXG - 1))

    nc.compile()
    return nc


def make_in_maps(input_, weight):
    X = np.asarray(input_, dtype=np.float32).reshape(M, H)
    W = np.asarray(weight, dtype=np.float32)
    in_maps = []
    for c in range(N_CORES):
        i, j = divmod(c, G_COL)
        # xH[g, p, k, mg] = X[i*M_LOC + g*XG + mg, k*P + p]
        xc = X[i * M_LOC:(i + 1) * M_LOC]                  # [M_LOC, H]
        xh = np.ascontiguousarray(
            xc.reshape(NXG, XG, KO, P).transpose(0, 3, 2, 1)
        )
        # wH[n, p, k, nq] = W[j*N_LOC + n*NT + nq, k*P + p]
        wc = W[j * N_LOC:(j + 1) * N_LOC]                  # [N_LOC, H]
        wh = np.ascontiguousarray(
            wc.reshape(NO, NT, KO, P).transpose(0, 3, 2, 1)
        )
        in_maps.append({"xH": xh, "wH": wh})
    return in_maps


def assemble(results):
    Y = np.empty((M, OUT), dtype=np.float32)
    for c in range(N_CORES):
        i, j = divmod(c, G_COL)
        Y[i * M_LOC:(i + 1) * M_LOC, j * N_LOC:(j + 1) * N_LOC] = results[c]["y"]
    return Y.reshape(S, B, OUT)


def kernel(input_, weight):
    nc = build_nc()
    res = run_bass_kernel_spmd(nc, make_in_maps(input_, weight), list(range(N_CORES)))
    return assemble(res.results)
